# revision 13
# baseline (speedup 1.0000x reference)
"""Trainium2 Bass kernel for nn_ConvAggregator (GNN FFT-conv aggregator).

Math: out = real(ifft2( prod_k fft2((feature @ W_aff + b_aff)[nbr_k]) )) @ W_mlp + b_mlp

Key transformation: fft2 of each 16x16 map is linear => fold the affine+FFT2
into one real matmul producing a packed 256-float spectrum per node
(Hermitian symmetry of real-input FFT: 126 conjugate-pair reps (Re,Im) +
4 self-conjugate real coefficients).  The K=16 neighbor reduction is an
elementwise complex product over packed spectra.  ifft2 + W_mlp fold into a
single [256,128] matmul on the packed product.

Sharding: data-parallel over destination nodes, 2500 nodes/core x 8 cores.
Every core redundantly computes the full spectrum table S [20000,256]
(cheap: one 128x256 matmul pass over features), then gathers its own
mailbox rows with GPSIMD dma_gather.
"""

import numpy as np

import concourse.bass as bass
import concourse.tile as tile
from concourse import bacc, mybir
from concourse.tile import add_dep_helper

F32 = mybir.dt.float32
I16 = mybir.dt.int16

H = 16
HID = 256          # H*H
SW = 256           # packed spectrum width
N = 20000
K = 16
IN_DIM = 128
OUT_DIM = 128
NCORES = 8
NPC = N // NCORES          # 2500 nodes per core
NPAD = 2560                # padded to 20 blocks of 128
ST = 512                   # nodes per supertile
NB = ST // 128             # node blocks per supertile (4)
NSUP = NPAD // ST          # 5
NIDX = ST * K              # 8192 gather indices per supertile
NGATH = NIDX // 1024       # dma_gather calls per supertile (HW SWDGE cap 1024)


# ----------------------------------------------------------------------------
# host-side constant matrices (packed FFT algebra)
# ----------------------------------------------------------------------------

def _build_rep_maps():
    seen, pairs, selfs = set(), [], []
    for u in range(H):
        for v in range(H):
            if (u, v) in seen:
                continue
            cu, cv = (-u) % H, (-v) % H
            if (cu, cv) == (u, v):
                selfs.append((u, v))
                seen.add((u, v))
            else:
                pairs.append((u, v))
                seen.add((u, v))
                seen.add((cu, cv))
    return pairs, selfs


def _build_matrices():
    """Tpack [256,256]: pre_flat -> packed ; Hmat [256,256]: packed -> h_flat.

    Packed layout: [Re(pair j) j<126 | self0 self1 | Im(pair j) j<126 | self2 self3]
    """
    pairs, selfs = _build_rep_maps()
    w = np.exp(-2j * np.pi * np.outer(np.arange(H), np.arange(H)) / H)
    F2D = np.kron(w, w)

    Tpack = np.zeros((HID, SW), dtype=np.float64)
    for j, (u, v) in enumerate(pairs):
        row = F2D[u * H + v]
        Tpack[:, j] = row.real
        Tpack[:, 128 + j] = row.imag
    for m, (u, v) in enumerate(selfs):
        col = 126 + m if m < 2 else 254 + (m - 2)
        Tpack[:, col] = F2D[u * H + v].real

    Hmat = np.zeros((SW, HID), dtype=np.float64)
    for t in range(SW):
        full = np.zeros(HID, dtype=np.complex128)
        if t < 126:
            u, v = pairs[t]
            full[u * H + v] += 1
            full[((-u) % H) * H + ((-v) % H)] += 1
        elif t < 128:
            u, v = selfs[t - 126]
            full[u * H + v] += 1
        elif t < 254:
            u, v = pairs[t - 128]
            full[u * H + v] += 1j
            full[((-u) % H) * H + ((-v) % H)] -= 1j
        else:
            u, v = selfs[2 + (t - 254)]
            full[u * H + v] += 1
        Hmat[t] = np.fft.ifft2(full.reshape(H, H)).real.flatten()
    return Tpack, Hmat


_TPACK, _HMAT = _build_matrices()


# ----------------------------------------------------------------------------
# bass module
# ----------------------------------------------------------------------------

def _cmul_step(nc, ppool, tpool, a, b):
    """One packed complex-multiply step: returns new P tile = a (*) b.

    a, b: APs of shape [128, NB, 256] (NB 128-node blocks, packed spectra).
    Slots (j, 128+j) j<126 complex; slots 126,127,254,255 plain reals.
    Re-half runs on DVE, Im-half on GPSIMD so the chain uses both engines.
    """
    pn = ppool.tile([128, NB, SW], F32, tag="pchain")
    aR, aI = a[:, :, 0:128], a[:, :, 128:256]
    bR, bI = b[:, :, 0:128], b[:, :, 128:256]
    t0 = tpool.tile([128, NB, 128], F32, tag="t0")
    t1 = tpool.tile([128, NB, 128], F32, tag="t1")
    nc.vector.tensor_tensor(t0[:], aR, bR, op=mybir.AluOpType.mult)
    nc.vector.tensor_tensor(t1[:], aI, bI, op=mybir.AluOpType.mult)
    nc.vector.tensor_tensor(pn[:, :, 0:128], t0[:], t1[:], op=mybir.AluOpType.subtract)
    t2 = tpool.tile([128, NB, 128], F32, tag="t2")
    t3 = tpool.tile([128, NB, 128], F32, tag="t3")
    nc.gpsimd.tensor_tensor(t2[:], aR, bI, op=mybir.AluOpType.mult)
    nc.gpsimd.tensor_tensor(t3[:], aI, bR, op=mybir.AluOpType.mult)
    nc.gpsimd.tensor_tensor(pn[:, :, 128:256], t2[:], t3[:], op=mybir.AluOpType.add)
    # fixup the 4 independent real slots (plain products)
    nc.vector.tensor_tensor(pn[:, :, 126:128], a[:, :, 126:128], b[:, :, 126:128],
                            op=mybir.AluOpType.mult)
    nc.vector.tensor_tensor(pn[:, :, 254:256], a[:, :, 254:256], b[:, :, 254:256],
                            op=mybir.AluOpType.mult)
    return pn


def build_module():
    from concourse.masks import make_identity

    nc = bacc.Bacc(None, name="conv_agg", target_bir_lowering=False)

    feat_t = nc.dram_tensor("feat_t", [IN_DIM, N], F32, kind="ExternalInput")
    wpack = nc.dram_tensor("wpack", [IN_DIM, SW], F32, kind="ExternalInput")
    bpack = nc.dram_tensor("bpack", [128, SW], F32, kind="ExternalInput")
    wpost = nc.dram_tensor("wpost", [SW, OUT_DIM], F32, kind="ExternalInput")
    bmlp = nc.dram_tensor("bmlp", [OUT_DIM, 1], F32, kind="ExternalInput")
    gidx = nc.dram_tensor("gidx", [NSUP, NGATH, 128, 64], I16, kind="ExternalInput")
    out_t = nc.dram_tensor("out_t", [OUT_DIM, NPAD], F32, kind="ExternalOutput")
    s_dram = nc.dram_tensor("s_spec", [N, SW], F32, kind="Internal")

    # phase-1 node chunks (128-wide, last ragged)
    chunks = []
    n0 = 0
    while n0 < N:
        m = min(128, N - n0)
        chunks.append((n0, m))
        n0 += m

    with tile.TileContext(nc) as tc:
        with tc.tile_pool(name="const", bufs=1) as cpool:
            wpack_sb = cpool.tile([IN_DIM, SW], F32)
            nc.sync.dma_start(out=wpack_sb[:], in_=wpack[:, :])
            bpk_sb = cpool.tile([128, SW], F32)
            nc.sync.dma_start(out=bpk_sb[:], in_=bpack[:, :])
            wpost_sb = cpool.tile([128, 2, OUT_DIM], F32)
            nc.sync.dma_start(out=wpost_sb[:, 0, :], in_=wpost[0:128, :])
            nc.sync.dma_start(out=wpost_sb[:, 1, :], in_=wpost[128:256, :])
            bmlp_sb = cpool.tile([OUT_DIM, 1], F32)
            nc.sync.dma_start(out=bmlp_sb[:], in_=bmlp[:, :])
            ident = cpool.tile([128, 128], F32)
            make_identity(nc, ident[:])

            # ---------------- phase 1: S = feature @ Wpack + bpack ----------
            s_writes = []
            GRP = 2048  # feature columns per load group
            with tc.tile_pool(name="p1f", bufs=3) as fpool, \
                 tc.tile_pool(name="p1s", bufs=4) as spool, \
                 tc.tile_pool(name="p1p", bufs=4, space="PSUM") as p1psum:
                g0 = 0
                while g0 < N:
                    gw = min(GRP, N - g0)
                    ft = fpool.tile([IN_DIM, GRP], F32, tag="ft")
                    nc.sync.dma_start(out=ft[:, :gw], in_=feat_t[:, g0:g0 + gw])
                    c0 = 0
                    while c0 < gw:
                        m = min(128, gw - c0)
                        ps = p1psum.tile([128, SW], F32, tag="ps")
                        nc.tensor.matmul(ps[:m, :], lhsT=ft[:, c0:c0 + m],
                                         rhs=wpack_sb[:], start=True, stop=True)
                        st = spool.tile([128, SW], F32, tag="st")
                        nc.vector.tensor_tensor(st[:m, :], ps[:m, :], bpk_sb[:m, :],
                                                op=mybir.AluOpType.add)
                        w = nc.sync.dma_start(out=s_dram[g0 + c0:g0 + c0 + m, :],
                                              in_=st[:m, :])
                        s_writes.append(w)
                        c0 += m
                    g0 += gw

            # join node: all S writes complete
            join = nc.sync.nop(nofuse=True, hint="phase1_done")
            for w in s_writes:
                add_dep_helper(join.ins, w.ins, reason="gather waits on S table")

            # ---------------- phase 2: gather + product + posttrans ---------
            with tc.tile_pool(name="p2i", bufs=2) as ipool, \
                 tc.tile_pool(name="p2m", bufs=2) as mpool, \
                 tc.tile_pool(name="p2p", bufs=3) as ppool, \
                 tc.tile_pool(name="p2t", bufs=2) as tpool, \
                 tc.tile_pool(name="p2x", bufs=4) as xpool, \
                 tc.tile_pool(name="p2o", bufs=3) as opool, \
                 tc.tile_pool(name="p2ps", bufs=4, space="PSUM") as p2psum:
                for s in range(NSUP):
                    mb = mpool.tile([128, NB * K, SW], F32, tag="mb")
                    for q in range(NGATH):
                        ix = ipool.tile([128, 64], I16, tag=f"ix{q}")
                        nc.sync.dma_start(out=ix[:], in_=gidx[s, q, :, :])
                        g = nc.gpsimd.dma_gather(mb[:, 8 * q:8 * (q + 1), :],
                                                 s_dram[:, :], ix[:],
                                                 1024, 1024, SW, elem_step=SW)
                        add_dep_helper(g.ins, join.ins, reason="gather after S ready")

                    p = _cmul_step(nc, ppool, tpool, mb[:, 0:NB, :],
                                   mb[:, NB:2 * NB, :])
                    for k in range(2, K):
                        p = _cmul_step(nc, ppool, tpool, p[:],
                                       mb[:, NB * k:NB * (k + 1), :])

                    for b in range(NB):
                        pts = []
                        for c in range(2):
                            ptp = p2psum.tile([128, 128], F32, tag="ptp")
                            nc.tensor.transpose(ptp[:], p[:, b, 128 * c:128 * (c + 1)],
                                                identity=ident[:])
                            pt = xpool.tile([128, 128], F32, tag=f"pt{c}")
                            nc.vector.tensor_copy(out=pt[:], in_=ptp[:])
                            pts.append(pt)
                        op_ps = p2psum.tile([128, 128], F32, tag="ops")
                        nc.tensor.matmul(op_ps[:], lhsT=wpost_sb[:, 0, :],
                                         rhs=pts[0][:], start=True, stop=False)
                        nc.tensor.matmul(op_ps[:], lhsT=wpost_sb[:, 1, :],
                                         rhs=pts[1][:], start=False, stop=True)
                        ob = opool.tile([OUT_DIM, 128], F32, tag="ob")
                        nc.scalar.activation(ob[:], op_ps[:],
                                             mybir.ActivationFunctionType.Identity,
                                             bias=bmlp_sb[:, 0:1], scale=1.0)
                        col = s * ST + b * 128
                        nc.sync.dma_start(out=out_t[:, col:col + 128], in_=ob[:])

    nc.compile()
    return nc


# ----------------------------------------------------------------------------
# host wrapper
# ----------------------------------------------------------------------------

_NC_CACHE = None


def _get_module():
    global _NC_CACHE
    if _NC_CACHE is None:
        _NC_CACHE = build_module()
    return _NC_CACHE


def _make_gidx(neighbors):
    """Per-core gather index tensors [NCORES, NSUP, 128, NIDX//16] int16."""
    nb = np.asarray(neighbors).astype(np.int64)
    out = np.zeros((NCORES, NSUP, NGATH, 128, 64), dtype=np.int16)
    for c in range(NCORES):
        nbp = np.zeros((NPAD, K), np.int64)
        nbp[:NPC] = nb[c * NPC:(c + 1) * NPC]
        for s in range(NSUP):
            blk = nbp[s * ST:(s + 1) * ST]              # [ST, 16]
            t = blk.reshape(NB, 128, K)                 # [b, p, k]
            flat = np.transpose(t, (2, 0, 1)).reshape(NIDX)   # i=(k*NB+b)*128+p
            for q in range(NGATH):
                fq = flat[q * 1024:(q + 1) * 1024]
                wrapped = fq.reshape(64, 16).T          # [16, 64]
                out[c, s, q] = np.tile(wrapped, (8, 1)).astype(np.int16)
    return out


def _make_inputs(feature, neighbors, W_aff, b_aff, W_mlp, b_mlp):
    feature = np.ascontiguousarray(np.asarray(feature, np.float32))
    Wpack = (np.asarray(W_aff, np.float64) @ _TPACK).astype(np.float32)
    bpack = (np.asarray(b_aff, np.float64) @ _TPACK).astype(np.float32)
    Wpost = (_HMAT @ np.asarray(W_mlp, np.float64)).astype(np.float32)
    feat_t = np.ascontiguousarray(feature.T)                       # [128, N]
    bpack_rep = np.ascontiguousarray(np.tile(bpack[None, :], (128, 1)))
    bmlp_col = np.ascontiguousarray(np.asarray(b_mlp, np.float32).reshape(OUT_DIM, 1))
    gidx = _make_gidx(neighbors)

    in_maps = []
    for c in range(NCORES):
        in_maps.append({
            "feat_t": feat_t,
            "wpack": Wpack,
            "bpack": bpack_rep,
            "wpost": np.ascontiguousarray(Wpost),
            "bmlp": bmlp_col,
            "gidx": np.ascontiguousarray(gidx[c]),
        })
    return in_maps


def kernel(feature, neighbors, W_aff, b_aff, W_mlp, b_mlp):
    from concourse import bass_utils

    nc = _get_module()
    in_maps = _make_inputs(feature, neighbors, W_aff, b_aff, W_mlp, b_mlp)
    res = bass_utils.run_bass_kernel_spmd(nc, in_maps, core_ids=list(range(NCORES)))
    out = np.empty((N, OUT_DIM), dtype=np.float32)
    for c in range(NCORES):
        out[c * NPC:(c + 1) * NPC] = res.results[c]["out_t"][:, :NPC].T
    return out


# revision 14
# speedup vs baseline: 1.0702x; 1.0702x over previous
"""Trainium2 Bass kernel for nn_ConvAggregator (GNN FFT-conv aggregator).

Math: out = real(ifft2( prod_k fft2((feature @ W_aff + b_aff)[nbr_k]) )) @ W_mlp + b_mlp

Key transformation: fft2 of each 16x16 map is linear => fold the affine+FFT2
into one real matmul producing a packed 256-float spectrum per node
(Hermitian symmetry of real-input FFT: 126 conjugate-pair reps (Re,Im) +
4 self-conjugate real coefficients).  The K=16 neighbor reduction is an
elementwise complex product over packed spectra.  ifft2 + W_mlp fold into a
single [256,128] matmul on the packed product.

Sharding: data-parallel over destination nodes, 2500 nodes/core x 8 cores.
Every core redundantly computes the full spectrum table S [20000,256]
(cheap: one 128x256 matmul pass over features), then gathers its own
mailbox rows with GPSIMD dma_gather.
"""

import numpy as np

import concourse.bass as bass
import concourse.tile as tile
from concourse import bacc, mybir
from concourse.tile import add_dep_helper

F32 = mybir.dt.float32
I16 = mybir.dt.int16

H = 16
HID = 256          # H*H
SW = 256           # packed spectrum width
N = 20000
K = 16
IN_DIM = 128
OUT_DIM = 128
NCORES = 8
NPC = N // NCORES          # 2500 nodes per core
NPAD = 2560                # padded to 20 blocks of 128
ST = 512                   # nodes per supertile
NB = ST // 128             # node blocks per supertile (4)
NSUP = NPAD // ST          # 5
NIDX = ST * K              # 8192 gather indices per supertile
NGATH = NIDX // 1024       # dma_gather calls per supertile (HW SWDGE cap 1024)


# ----------------------------------------------------------------------------
# host-side constant matrices (packed FFT algebra)
# ----------------------------------------------------------------------------

def _build_rep_maps():
    seen, pairs, selfs = set(), [], []
    for u in range(H):
        for v in range(H):
            if (u, v) in seen:
                continue
            cu, cv = (-u) % H, (-v) % H
            if (cu, cv) == (u, v):
                selfs.append((u, v))
                seen.add((u, v))
            else:
                pairs.append((u, v))
                seen.add((u, v))
                seen.add((cu, cv))
    return pairs, selfs


def _build_matrices():
    """Tpack [256,256]: pre_flat -> packed ; Hmat [256,256]: packed -> h_flat.

    Packed layout: [Re(pair j) j<126 | self0 self1 | Im(pair j) j<126 | self2 self3]
    """
    pairs, selfs = _build_rep_maps()
    w = np.exp(-2j * np.pi * np.outer(np.arange(H), np.arange(H)) / H)
    F2D = np.kron(w, w)

    Tpack = np.zeros((HID, SW), dtype=np.float64)
    for j, (u, v) in enumerate(pairs):
        row = F2D[u * H + v]
        Tpack[:, j] = row.real
        Tpack[:, 128 + j] = row.imag
    for m, (u, v) in enumerate(selfs):
        col = 126 + m if m < 2 else 254 + (m - 2)
        Tpack[:, col] = F2D[u * H + v].real

    Hmat = np.zeros((SW, HID), dtype=np.float64)
    for t in range(SW):
        full = np.zeros(HID, dtype=np.complex128)
        if t < 126:
            u, v = pairs[t]
            full[u * H + v] += 1
            full[((-u) % H) * H + ((-v) % H)] += 1
        elif t < 128:
            u, v = selfs[t - 126]
            full[u * H + v] += 1
        elif t < 254:
            u, v = pairs[t - 128]
            full[u * H + v] += 1j
            full[((-u) % H) * H + ((-v) % H)] -= 1j
        else:
            u, v = selfs[2 + (t - 254)]
            full[u * H + v] += 1
        Hmat[t] = np.fft.ifft2(full.reshape(H, H)).real.flatten()
    return Tpack, Hmat


_TPACK, _HMAT = _build_matrices()


# ----------------------------------------------------------------------------
# bass module
# ----------------------------------------------------------------------------

def _cmul_step(nc, ppool, tpool, a, b):
    """One packed complex-multiply step: returns new P tile = a (*) b.

    a, b: APs of shape [128, NB, 256] (NB 128-node blocks, packed spectra).
    Slots (j, 128+j) j<126 complex; slots 126,127,254,255 plain reals.
    Re-half runs on DVE, Im-half on GPSIMD so the chain uses both engines.
    """
    pn = ppool.tile([128, NB, SW], F32, tag="pchain")
    aR, aI = a[:, :, 0:128], a[:, :, 128:256]
    bR, bI = b[:, :, 0:128], b[:, :, 128:256]
    t0 = tpool.tile([128, NB, 128], F32, tag="t0")
    t1 = tpool.tile([128, NB, 128], F32, tag="t1")
    nc.vector.tensor_tensor(t0[:], aR, bR, op=mybir.AluOpType.mult)
    nc.vector.tensor_tensor(t1[:], aI, bI, op=mybir.AluOpType.mult)
    nc.vector.tensor_tensor(pn[:, :, 0:128], t0[:], t1[:], op=mybir.AluOpType.subtract)
    t2 = tpool.tile([128, NB, 128], F32, tag="t2")
    t3 = tpool.tile([128, NB, 128], F32, tag="t3")
    nc.gpsimd.tensor_tensor(t2[:], aR, bI, op=mybir.AluOpType.mult)
    nc.gpsimd.tensor_tensor(t3[:], aI, bR, op=mybir.AluOpType.mult)
    nc.gpsimd.tensor_tensor(pn[:, :, 128:256], t2[:], t3[:], op=mybir.AluOpType.add)
    # fixup the 4 independent real slots (plain products)
    nc.vector.tensor_tensor(pn[:, :, 126:128], a[:, :, 126:128], b[:, :, 126:128],
                            op=mybir.AluOpType.mult)
    nc.vector.tensor_tensor(pn[:, :, 254:256], a[:, :, 254:256], b[:, :, 254:256],
                            op=mybir.AluOpType.mult)
    return pn


def build_module():
    from concourse.masks import make_identity

    nc = bacc.Bacc(None, name="conv_agg", target_bir_lowering=False)

    feat_t = nc.dram_tensor("feat_t", [IN_DIM, N], F32, kind="ExternalInput")
    wpack = nc.dram_tensor("wpack", [IN_DIM, SW], F32, kind="ExternalInput")
    bpack = nc.dram_tensor("bpack", [128, SW], F32, kind="ExternalInput")
    wpost = nc.dram_tensor("wpost", [SW, OUT_DIM], F32, kind="ExternalInput")
    bmlp = nc.dram_tensor("bmlp", [OUT_DIM, 1], F32, kind="ExternalInput")
    gidx = nc.dram_tensor("gidx", [NSUP, NGATH, 128, 64], I16, kind="ExternalInput")
    out_t = nc.dram_tensor("out_t", [OUT_DIM, NPAD], F32, kind="ExternalOutput")
    s_dram = nc.dram_tensor("s_spec", [N, SW], F32, kind="Internal")

    # phase-1 node chunks (128-wide, last ragged)
    chunks = []
    n0 = 0
    while n0 < N:
        m = min(128, N - n0)
        chunks.append((n0, m))
        n0 += m

    with tile.TileContext(nc) as tc:
        with tc.tile_pool(name="const", bufs=1) as cpool:
            wpack_sb = cpool.tile([IN_DIM, SW], F32)
            nc.sync.dma_start(out=wpack_sb[:], in_=wpack[:, :])
            bpk_sb = cpool.tile([128, SW], F32)
            nc.sync.dma_start(out=bpk_sb[:], in_=bpack[:, :])
            wpost_sb = cpool.tile([128, 2, OUT_DIM], F32)
            nc.sync.dma_start(out=wpost_sb[:, 0, :], in_=wpost[0:128, :])
            nc.sync.dma_start(out=wpost_sb[:, 1, :], in_=wpost[128:256, :])
            bmlp_sb = cpool.tile([OUT_DIM, 1], F32)
            nc.sync.dma_start(out=bmlp_sb[:], in_=bmlp[:, :])
            ident = cpool.tile([128, 128], F32)
            make_identity(nc, ident[:])

            # ---------------- phase 1: S = feature @ Wpack + bpack ----------
            s_writes = []
            GRP = 2048  # feature columns per load group
            with tc.tile_pool(name="p1f", bufs=3) as fpool, \
                 tc.tile_pool(name="p1s", bufs=4) as spool, \
                 tc.tile_pool(name="p1p", bufs=4, space="PSUM") as p1psum:
                g0 = 0
                while g0 < N:
                    gw = min(GRP, N - g0)
                    ft = fpool.tile([IN_DIM, GRP], F32, tag="ft")
                    nc.sync.dma_start(out=ft[:, :gw], in_=feat_t[:, g0:g0 + gw])
                    if gw == GRP:
                        # full group: stage 8 chunks, write 1MB batched DMAs
                        for h in range(2):
                            sg = spool.tile([128, 8, SW], F32, tag="sg")
                            for i in range(8):
                                c0 = h * 1024 + i * 128
                                ps = p1psum.tile([128, SW], F32, tag="ps")
                                nc.tensor.matmul(ps[:], lhsT=ft[:, c0:c0 + 128],
                                                 rhs=wpack_sb[:], start=True,
                                                 stop=True)
                                nc.vector.tensor_tensor(
                                    sg[:, i, :], ps[:], bpk_sb[:],
                                    op=mybir.AluOpType.add)
                            dst = s_dram[g0 + h * 1024:g0 + (h + 1) * 1024, :]
                            w = nc.sync.dma_start(
                                out=dst.rearrange("(c p) e -> p c e", p=128),
                                in_=sg[:])
                            s_writes.append(w)
                    else:
                        # ragged tail: per-chunk writes
                        c0 = 0
                        while c0 < gw:
                            m = min(128, gw - c0)
                            ps = p1psum.tile([128, SW], F32, tag="ps")
                            nc.tensor.matmul(ps[:m, :], lhsT=ft[:, c0:c0 + m],
                                             rhs=wpack_sb[:], start=True, stop=True)
                            st = spool.tile([128, SW], F32, tag="st")
                            nc.vector.tensor_tensor(st[:m, :], ps[:m, :],
                                                    bpk_sb[:m, :],
                                                    op=mybir.AluOpType.add)
                            w = nc.sync.dma_start(
                                out=s_dram[g0 + c0:g0 + c0 + m, :], in_=st[:m, :])
                            s_writes.append(w)
                            c0 += m
                    g0 += gw

            # join node: all S writes complete
            join = nc.sync.nop(nofuse=True, hint="phase1_done")
            for w in s_writes:
                add_dep_helper(join.ins, w.ins, reason="gather waits on S table")

            # ---------------- phase 2: gather + product + posttrans ---------
            with tc.tile_pool(name="p2i", bufs=2) as ipool, \
                 tc.tile_pool(name="p2m", bufs=2) as mpool, \
                 tc.tile_pool(name="p2p", bufs=3) as ppool, \
                 tc.tile_pool(name="p2t", bufs=2) as tpool, \
                 tc.tile_pool(name="p2x", bufs=4) as xpool, \
                 tc.tile_pool(name="p2o", bufs=3) as opool, \
                 tc.tile_pool(name="p2ps", bufs=4, space="PSUM") as p2psum:
                for s in range(NSUP):
                    mb = mpool.tile([128, NB * K, SW], F32, tag="mb")
                    for q in range(NGATH):
                        ix = ipool.tile([128, 64], I16, tag=f"ix{q}")
                        nc.sync.dma_start(out=ix[:], in_=gidx[s, q, :, :])
                        g = nc.gpsimd.dma_gather(mb[:, 8 * q:8 * (q + 1), :],
                                                 s_dram[:, :], ix[:],
                                                 1024, 1024, SW, elem_step=SW)
                        add_dep_helper(g.ins, join.ins, reason="gather after S ready")

                    p = _cmul_step(nc, ppool, tpool, mb[:, 0:NB, :],
                                   mb[:, NB:2 * NB, :])
                    for k in range(2, K):
                        p = _cmul_step(nc, ppool, tpool, p[:],
                                       mb[:, NB * k:NB * (k + 1), :])

                    for b in range(NB):
                        pts = []
                        for c in range(2):
                            ptp = p2psum.tile([128, 128], F32, tag="ptp")
                            nc.tensor.transpose(ptp[:], p[:, b, 128 * c:128 * (c + 1)],
                                                identity=ident[:])
                            pt = xpool.tile([128, 128], F32, tag=f"pt{c}")
                            nc.vector.tensor_copy(out=pt[:], in_=ptp[:])
                            pts.append(pt)
                        op_ps = p2psum.tile([128, 128], F32, tag="ops")
                        nc.tensor.matmul(op_ps[:], lhsT=wpost_sb[:, 0, :],
                                         rhs=pts[0][:], start=True, stop=False)
                        nc.tensor.matmul(op_ps[:], lhsT=wpost_sb[:, 1, :],
                                         rhs=pts[1][:], start=False, stop=True)
                        ob = opool.tile([OUT_DIM, 128], F32, tag="ob")
                        nc.scalar.activation(ob[:], op_ps[:],
                                             mybir.ActivationFunctionType.Identity,
                                             bias=bmlp_sb[:, 0:1], scale=1.0)
                        col = s * ST + b * 128
                        nc.sync.dma_start(out=out_t[:, col:col + 128], in_=ob[:])

    nc.compile()
    return nc


# ----------------------------------------------------------------------------
# host wrapper
# ----------------------------------------------------------------------------

_NC_CACHE = None


def _get_module():
    global _NC_CACHE
    if _NC_CACHE is None:
        _NC_CACHE = build_module()
    return _NC_CACHE


def _make_gidx(neighbors):
    """Per-core gather index tensors [NCORES, NSUP, 128, NIDX//16] int16."""
    nb = np.asarray(neighbors).astype(np.int64)
    out = np.zeros((NCORES, NSUP, NGATH, 128, 64), dtype=np.int16)
    for c in range(NCORES):
        nbp = np.zeros((NPAD, K), np.int64)
        nbp[:NPC] = nb[c * NPC:(c + 1) * NPC]
        for s in range(NSUP):
            blk = nbp[s * ST:(s + 1) * ST]              # [ST, 16]
            t = blk.reshape(NB, 128, K)                 # [b, p, k]
            flat = np.transpose(t, (2, 0, 1)).reshape(NIDX)   # i=(k*NB+b)*128+p
            for q in range(NGATH):
                fq = flat[q * 1024:(q + 1) * 1024]
                wrapped = fq.reshape(64, 16).T          # [16, 64]
                out[c, s, q] = np.tile(wrapped, (8, 1)).astype(np.int16)
    return out


def _make_inputs(feature, neighbors, W_aff, b_aff, W_mlp, b_mlp):
    feature = np.ascontiguousarray(np.asarray(feature, np.float32))
    Wpack = (np.asarray(W_aff, np.float64) @ _TPACK).astype(np.float32)
    bpack = (np.asarray(b_aff, np.float64) @ _TPACK).astype(np.float32)
    Wpost = (_HMAT @ np.asarray(W_mlp, np.float64)).astype(np.float32)
    feat_t = np.ascontiguousarray(feature.T)                       # [128, N]
    bpack_rep = np.ascontiguousarray(np.tile(bpack[None, :], (128, 1)))
    bmlp_col = np.ascontiguousarray(np.asarray(b_mlp, np.float32).reshape(OUT_DIM, 1))
    gidx = _make_gidx(neighbors)

    in_maps = []
    for c in range(NCORES):
        in_maps.append({
            "feat_t": feat_t,
            "wpack": Wpack,
            "bpack": bpack_rep,
            "wpost": np.ascontiguousarray(Wpost),
            "bmlp": bmlp_col,
            "gidx": np.ascontiguousarray(gidx[c]),
        })
    return in_maps


def kernel(feature, neighbors, W_aff, b_aff, W_mlp, b_mlp):
    from concourse import bass_utils

    nc = _get_module()
    in_maps = _make_inputs(feature, neighbors, W_aff, b_aff, W_mlp, b_mlp)
    res = bass_utils.run_bass_kernel_spmd(nc, in_maps, core_ids=list(range(NCORES)))
    out = np.empty((N, OUT_DIM), dtype=np.float32)
    for c in range(NCORES):
        out[c * NPC:(c + 1) * NPC] = res.results[c]["out_t"][:, :NPC].T
    return out
